# revision 33
# baseline (speedup 1.0000x reference)
"""GCN critic (2x GCNConv + 2 MLP heads) on 8 trn2 NeuronCores.

Sharding: destination-node blocks of 1250 nodes per core. Edges bucketed by
dst window (128 dst nodes). conv1 gathers raw (dis-scaled) input features
(256B rows) from a replicated table; the w1 matmul is applied after the
segment-sum (linearity). conv2 gathers 256B rows of dis*relu(out1) from
AllGather'ed tables. Segment-sum is done per 128-edge chunk with a one-hot
matmul on the tensor engine (S[e,d] = (dst[e]==d)); msg.T @ S accumulates
feature-major segments in PSUM.

Key optimizations over the naive pipeline:
- one-hot S tiles are built once on DVE and shared by both convs;
- all dense matmuls (W1/W2/heads) run in bf16; heads use per-partition
  bias via the scalar engine plus a 1-column matmul for the final dot;
- windows are processed in pairs with one msg tile and one gather batch
  per pair, halving the number of partial (sub-1024-index) dma_gathers;
- the inter-conv AllGather is split in two; node ids are remapped
  host-side to the core-interleaved order the split AllGather produces;
  conv2's edges are further split per window into an A part (src row in
  the first AllGather half) and a B part, gathered from per-half tables
  so A gathers overlap the second collective;
- q outputs are transposed on the PE and written with 2 DMAs per head.
"""

import numpy as np
import ml_dtypes

BF16 = ml_dtypes.bfloat16
N_NODES = 10000
OBS_DIM = 30
ACT_DIM = 4
HID = 128
N_CORES = 8
BLK = N_NODES // N_CORES  # 1250 dst nodes per core
P = 128
NWIN = (BLK + P - 1) // P  # 10 windows per core (last is 98 wide)
GMAX = 2048  # max idx per dma_gather instruction
XCOLS = 128  # conv1 gather row (bf16): 34 used, pad to 256B
SPLIT = 4 * P  # rows 0..512 of each core strip go in AllGather #1
SPLIT2 = BLK - SPLIT  # 610


def _remap(node_ids):
    """Node id -> row in the split-AllGather table layout.

    AllGather #1 concatenates core strips' rows [0, SPLIT); #2 the rest.
    """
    c, r = node_ids // BLK, node_ids % BLK
    return np.where(r < SPLIT, c * SPLIT + r,
                    N_CORES * SPLIT + c * SPLIT2 + (r - SPLIT))


def _prep_graph(edge_index):
    """Host-side index preprocessing (the sharding step).

    Edges are bucketed by (core, dst window), then split into an A part
    (remapped src row < N_CORES*SPLIT, i.e. covered by AllGather #1) and a
    B part, each padded to 128-edge chunks. Chunk layout per window pair
    (w0, w1): [A(w0) | A(w1) | B(w0) | B(w1)] so that each pair needs one
    contiguous A gather batch and one contiguous B gather batch.
    """
    NA = N_CORES * SPLIT
    src = np.asarray(edge_index[0], dtype=np.int64)
    dst = np.asarray(edge_index[1], dtype=np.int64)
    loops = np.arange(N_NODES, dtype=np.int64)
    src = np.concatenate([src, loops])
    dst = np.concatenate([dst, loops])
    deg = np.bincount(dst, minlength=N_NODES).astype(np.float32)
    dis = (1.0 / np.sqrt(np.maximum(deg, 1.0))).astype(np.float32)

    src_m = _remap(src)
    isB = (src_m >= NA).astype(np.int64)
    win = (dst // BLK) * NWIN + (dst % BLK) // P  # (core, local window)
    nwin_g = N_CORES * NWIN
    order = np.lexsort((src_m, isB, win))
    src_s, dst_s, win_s, isB_s = src_m[order], dst[order], win[order], isB[order]
    grp = win_s * 2 + isB_s
    counts = np.bincount(grp, minlength=nwin_g * 2)
    starts = np.concatenate([[0], np.cumsum(counts)])[:-1]

    # common per-(window, part) chunk counts across cores (SPMD)
    chunks_wA = np.zeros(NWIN, dtype=np.int64)
    chunks_wB = np.zeros(NWIN, dtype=np.int64)
    for w in range(NWIN):
        chunks_wA[w] = (max(counts[(c * NWIN + w) * 2] for c in range(N_CORES)) + P - 1) // P
        chunks_wB[w] = (max(counts[(c * NWIN + w) * 2 + 1] for c in range(N_CORES)) + P - 1) // P

    tot_chunks = int(chunks_wA.sum() + chunks_wB.sum())
    tot_e = tot_chunks * P
    idx_all = np.zeros((N_CORES, tot_e), np.int16)
    dstc_all = np.full((N_CORES, tot_e), -1.0, np.float32)
    for c in range(N_CORES):
        off = 0
        for w0 in range(0, NWIN, 2):
            pair = [w for w in (w0, w0 + 1) if w < NWIN]
            for part, padidx, chunks_p in ((0, 0, chunks_wA), (1, NA, chunks_wB)):
                for w in pair:
                    g = (c * NWIN + w) * 2 + part
                    n = counts[g]
                    s0 = starts[g]
                    idx_all[c, off:off + chunks_p[w] * P] = padidx
                    idx_all[c, off:off + n] = src_s[s0:s0 + n].astype(np.int16)
                    dstc_all[c, off:off + n] = (dst_s[s0:s0 + n] - (c * BLK + w * P)).astype(np.float32)
                    off += chunks_p[w] * P
    # wrap idx: position i -> partition i%16, col i//16; replicate to 8 groups
    pos = np.arange(tot_e)
    idx_wrap = np.zeros((N_CORES, P, tot_e // 16), np.int16)
    for g in range(8):
        idx_wrap[:, g * 16 + pos % 16, pos // 16] = idx_all
    # dst cols: chunk k partition e
    dstc = dstc_all.reshape(N_CORES, tot_chunks, P).transpose(0, 2, 1).copy()
    return idx_wrap, dstc, chunks_wA, chunks_wB, dis


def _build(chunks_wA, chunks_wB):
    import concourse.bacc as bacc
    import concourse.mybir as mybir
    from concourse.tile import TileContext
    from concourse import library_config

    dt = mybir.dt
    act = mybir.ActivationFunctionType
    tot_chunks = int(chunks_wA.sum() + chunks_wB.sum())
    tot_e = tot_chunks * P

    nc = bacc.Bacc(None, target_bir_lowering=False, num_devices=N_CORES,
                   num_swdge_queues=4)
    # ---- inputs ----
    # bf16 pack cols: iota(P) | ident(P) | w1p(HID) | w2(HID) | wq1a(HID)
    #                | wq2a(HID) | w1bb(1) | w2bb(1)
    NB = 6 * P + 2
    # fp32 pack cols: identf(P) | b1(1) | b2(1) | a1c(1) | a2c(1) | bq(2)
    NF = P + 6
    x_dis = nc.dram_tensor("x_dis", [N_NODES, XCOLS], dt.bfloat16, kind="ExternalInput")
    idx_in = nc.dram_tensor("idx", [P, tot_e // 16], dt.int16, kind="ExternalInput")
    dstc_in = nc.dram_tensor("dstc", [P, tot_chunks], dt.bfloat16, kind="ExternalInput")
    bpk_in = nc.dram_tensor("bpk", [P, NB], dt.bfloat16, kind="ExternalInput")
    fpk_in = nc.dram_tensor("fpk", [P, NF], dt.float32, kind="ExternalInput")
    disb_in = nc.dram_tensor("disb", [P, NWIN * P], dt.float32, kind="ExternalInput")
    q1_out = nc.dram_tensor("q1", [BLK, 1], dt.float32, kind="ExternalOutput")
    q2_out = nc.dram_tensor("q2", [BLK, 1], dt.float32, kind="ExternalOutput")
    # per-AllGather-half gather tables (full-size so src ids stay absolute)
    x2d_fullA = nc.dram_tensor("x2d_fullA", [N_NODES, HID], dt.bfloat16,
                               kind="Internal", addr_space="Shared")
    x2d_fullB = nc.dram_tensor("x2d_fullB", [N_NODES, HID], dt.bfloat16,
                               kind="Internal", addr_space="Shared")

    with TileContext(nc) as tc:
        nc.gpsimd.load_library(library_config.mlp)
        with tc.tile_pool(name="const", bufs=1) as cp, \
             tc.tile_pool(name="msgp", bufs=3) as msgp, \
             tc.tile_pool(name="work", bufs=2) as wp, \
             tc.tile_pool(name="psum", bufs=4, space="PSUM") as pp, \
             tc.tile_pool(name="psum2", bufs=2, space="PSUM") as pp2, \
             tc.tile_pool(name="dram", bufs=1, space="DRAM") as dramp:

            # ---- load constants ----
            idx_t = cp.tile([P, tot_e // 16], dt.int16)
            nc.sync.dma_start(idx_t[:], idx_in[:])
            dstc_t = cp.tile([P, tot_chunks], dt.bfloat16)
            nc.scalar.dma_start(dstc_t[:], dstc_in[:])
            bpk_t = cp.tile([P, NB], dt.bfloat16)
            nc.scalar.dma_start(bpk_t[:], bpk_in[:])
            fpk_t = cp.tile([P, NF], dt.float32)
            nc.scalar.dma_start(fpk_t[:], fpk_in[:])
            disb_t = cp.tile([P, NWIN * P], dt.float32)
            nc.scalar.dma_start(disb_t[:], disb_in[:])

            iota_t = bpk_t[:, 0 * P:1 * P]
            ident_t = bpk_t[:, 1 * P:2 * P]
            w1_t = bpk_t[:, 2 * P:3 * P]
            w2_t = bpk_t[:, 3 * P:4 * P]
            wq1a_t = bpk_t[:, 4 * P:5 * P]
            wq2a_t = bpk_t[:, 5 * P:6 * P]
            w1bb_t = bpk_t[:, 6 * P:6 * P + 1]
            w2bb_t = bpk_t[:, 6 * P + 1:6 * P + 2]
            identf_t = fpk_t[:, 0:P]
            b1_t = fpk_t[:, P + 0:P + 1]
            b2_t = fpk_t[:, P + 1:P + 2]
            a1c_t = fpk_t[:, P + 2:P + 3]
            a2c_t = fpk_t[:, P + 3:P + 4]
            bq1_t = fpk_t[:, P + 4:P + 5]
            bq2_t = fpk_t[:, P + 5:P + 6]

            # split so AllGather #1's dependency covers only windows 0-4
            x2d_lo = dramp.tile([SPLIT, HID], dt.bfloat16)
            x2d_hi = dramp.tile([SPLIT2, HID], dt.bfloat16)
            q1_col = cp.tile([P, NWIN], dt.float32)
            q2_col = cp.tile([P, NWIN], dt.float32)

            qn = [0]

            # ---- one-hot S tiles: built once on DVE, shared by both convs ----
            S_all = cp.tile([P, tot_chunks, P], dt.bfloat16)
            SB = 34
            for c0 in range(0, tot_chunks, SB):
                nchunks = min(SB, tot_chunks - c0)
                dcol = dstc_t[:, c0:c0 + nchunks].rearrange(
                    "p (k o) -> p k o", o=1).broadcast_to([P, nchunks, P])
                irow = iota_t.rearrange(
                    "p (o d) -> p o d", o=1).broadcast_to([P, nchunks, P])
                nc.vector.tensor_tensor(out=S_all[:, c0:c0 + nchunks, :],
                                        in0=dcol, in1=irow,
                                        op=mybir.AluOpType.is_equal)

            def gather_span(table, msg, koff, c0, nchunks, ecols):
                """Issue dma_gathers for nchunks*P edges into msg at koff."""
                e0 = c0 * P
                n_left = nchunks * P
                off = 0
                while n_left > 0:
                    g = min(n_left, GMAX)
                    nc.gpsimd.dma_gather(
                        out_ap=msg[:, koff + off // P:koff + (off + g) // P, :],
                        in_ap=table[:],
                        idxs_ap=idx_t[:, (e0 + off) // 16:(e0 + off + g) // 16],
                        num_idxs=g, num_idxs_reg=g, elem_size=ecols,
                        queue_num=qn[0] % 4,
                    )
                    qn[0] += 1
                    off += g
                    n_left -= g

            def scatter_ranges(msg, ranges, ecols):
                """One-hot matmul segment sum over chunk ranges -> psum
                [ecols, P] feature-major. ranges: list of (koff, c0, n)."""
                seg = pp.tile([ecols, P], dt.float32, space="PSUM", tag="seg")
                total = sum(r[2] for r in ranges)
                i = 0
                for (koff, c0, n) in ranges:
                    for k in range(n):
                        nc.tensor.matmul(out=seg[:], lhsT=msg[:, koff + k, :],
                                         rhs=S_all[:, c0 + k, :],
                                         start=(i == 0), stop=(i == total - 1))
                        i += 1
                return seg

            def heads(x3, w):
                """q heads: h = relu(wqa.T @ x3 + a); q[d] = h.T @ wbb + bq."""
                for (wqa_t, ac_t, wbb_t, qcol, bqc) in (
                        (wq1a_t, a1c_t, w1bb_t, q1_col, bq1_t),
                        (wq2a_t, a2c_t, w2bb_t, q2_col, bq2_t)):
                    hp = pp2.tile([HID, P], dt.float32, space="PSUM", tag="mm")
                    nc.tensor.matmul(out=hp[:], lhsT=wqa_t, rhs=x3[:],
                                     start=True, stop=True)  # [f', d]
                    hr = wp.tile([HID, P], dt.bfloat16, tag="hr")
                    nc.scalar.activation(hr[:], hp[:], act.Relu,
                                         bias=ac_t, scale=1.0)
                    qp = pp2.tile([P, 1], dt.float32, space="PSUM", tag="qp")
                    nc.tensor.matmul(out=qp[:], lhsT=hr[:], rhs=wbb_t,
                                     start=True, stop=True)  # [d, 1]
                    nc.vector.tensor_scalar(out=qcol[:, w:w + 1], in0=qp[:],
                                            scalar1=bqc, scalar2=None,
                                            op0=mybir.AluOpType.add)

            # ================= conv1 =================
            # windows processed in pairs; chunk layout per pair is
            # [A(w0) | A(w1) | B(w0) | B(w1)] (A: src row < N_CORES*SPLIT)
            def pair_ranges(pair, c0):
                kA = [int(chunks_wA[w]) for w in pair]
                kB = [int(chunks_wB[w]) for w in pair]
                nA = sum(kA)
                r = {}
                offs = [0, kA[0], nA, nA + kB[0]]
                for j, w in enumerate(pair):
                    r[w] = [(offs[j], c0 + offs[j], kA[j]),
                            (offs[2 + j], c0 + offs[2 + j], kB[j])]
                return nA, sum(kB), r

            c0 = 0
            x2d_sb = cp.tile([P, NWIN, HID], dt.bfloat16)  # node-major x2d blocks
            for w0 in range(0, NWIN, 2):
              pair = [w for w in (w0, w0 + 1) if w < NWIN]
              nA, nB, pranges = pair_ranges(pair, c0)
              nch_pair = nA + nB
              msg = msgp.tile([P, nch_pair, XCOLS], dt.bfloat16, tag="msg")
              gather_span(x_dis, msg, 0, c0, nch_pair, XCOLS)
              for w in pair:
                wlen = min(P, BLK - w * P)
                segx = scatter_ranges(msg, pranges[w], XCOLS)  # psum fm
                segx_sb = wp.tile([XCOLS, P], dt.bfloat16, tag="segx")
                nc.scalar.copy(segx_sb[:], segx[:])
                o1 = pp2.tile([HID, P], dt.float32, space="PSUM", tag="mm")
                nc.tensor.matmul(out=o1[:], lhsT=w1_t, rhs=segx_sb[:],
                                 start=True, stop=True)  # [128f, 128d] fm
                t1 = wp.tile([HID, P], dt.float32, tag="t1")
                nc.vector.tensor_mul(t1[:], o1[:], disb_t[:, w * P:w * P + P])
                x2 = wp.tile([HID, P], dt.float32, tag="x2")
                nc.scalar.activation(x2[:], t1[:], act.Relu, bias=b1_t, scale=1.0)
                x2d = wp.tile([HID, P], dt.bfloat16, tag="x2d")
                nc.vector.tensor_mul(x2d[:], x2[:], disb_t[:, w * P:w * P + P])
                # transpose to node-major and stash
                x2d_tp = pp2.tile([P, HID], dt.bfloat16, space="PSUM", tag="mm")
                nc.tensor.transpose(out=x2d_tp[:], in_=x2d[:], identity=ident_t)
                nc.scalar.copy(x2d_sb[:, w, :], x2d_tp[:])
                if w < SPLIT // P:
                    nc.sync.dma_start(x2d_lo[w * P:w * P + wlen, :], x2d_sb[:wlen, w, :])
                else:
                    r0 = w * P - SPLIT
                    nc.sync.dma_start(x2d_hi[r0:r0 + wlen, :], x2d_sb[:wlen, w, :])
                if w == 5:  # x2d_lo written ~here; doorbell placed late to avoid a SEQ stall
                    nc.gpsimd.collective_compute(
                        "AllGather", mybir.AluOpType.bypass,
                        replica_groups=[list(range(N_CORES))],
                        ins=[x2d_lo[:].opt()],
                        outs=[x2d_fullA[0:N_CORES * SPLIT, :].opt()])
                    # scheduler fence: keep the collective's doorbell ahead
                    # of windows 5-9's gathers in the gpsimd stream so it
                    # fires mid-conv1 (ordering only, no semaphore waits)
                    tc.no_sync_barrier()
              c0 += nch_pair

            # ================= exchange (second half) =================
            nc.gpsimd.collective_compute(
                "AllGather", mybir.AluOpType.bypass,
                replica_groups=[list(range(N_CORES))],
                ins=[x2d_hi[:].opt()],
                outs=[x2d_fullB[N_CORES * SPLIT:N_NODES, :].opt()])

            # ================= conv2 + heads =================
            # A chunks gather from the first AllGather half's table (ready
            # before the second collective completes); B from the second
            c0 = 0
            for w0 in range(0, NWIN, 2):
              pair = [w for w in (w0, w0 + 1) if w < NWIN]
              nA, nB, pranges = pair_ranges(pair, c0)
              nch_pair = nA + nB
              msg = msgp.tile([P, nch_pair, HID], dt.bfloat16, tag="msg")
              gather_span(x2d_fullA, msg, 0, c0, nA, HID)
              gather_span(x2d_fullB, msg, nA, c0 + nA, nB, HID)
              for w in pair:
                seg2 = scatter_ranges(msg, pranges[w], HID)  # psum fm
                seg2_sb = wp.tile([HID, P], dt.bfloat16, tag="seg2")
                nc.scalar.copy(seg2_sb[:], seg2[:])
                o2 = pp2.tile([HID, P], dt.float32, space="PSUM", tag="mm")
                nc.tensor.matmul(out=o2[:], lhsT=w2_t, rhs=seg2_sb[:],
                                 start=True, stop=True)
                t2 = wp.tile([HID, P], dt.float32, tag="t2")
                nc.vector.tensor_mul(t2[:], o2[:], disb_t[:, w * P:w * P + P])
                x3 = wp.tile([HID, P], dt.bfloat16, tag="x3")
                nc.scalar.activation(x3[:], t2[:], act.Relu, bias=b2_t, scale=1.0)
                heads(x3, w)
              c0 += nch_pair

            # transpose q columns to window-major and write with 2 DMAs per head
            for (qcol, q_out) in ((q1_col, q1_out), (q2_col, q2_out)):
                qtp = pp2.tile([NWIN, P], dt.float32, space="PSUM", tag="mm")
                nc.tensor.transpose(out=qtp[:], in_=qcol[:], identity=identf_t)
                qsb = wp.tile([NWIN, P], dt.float32, tag="qsb")
                nc.scalar.copy(qsb[:], qtp[:])
                nfull = (BLK // P) * P  # 1152
                nc.sync.dma_start(
                    q_out[0:nfull, :].rearrange("(w p) o -> w (p o)", p=P),
                    qsb[0:BLK // P, :])
                nc.sync.dma_start(
                    q_out[nfull:BLK, :].rearrange("(w p) o -> w (p o)", w=1),
                    qsb[BLK // P:BLK // P + 1, 0:BLK - nfull])

    nc.compile()
    return nc


_CACHE = {}


def kernel(obs, action, edge_index,
           w_g1, b_g1, w_g2, b_g2,
           w_q1a, b_q1a, w_q1b, b_q1b,
           w_q2a, b_q2a, w_q2b, b_q2b, _trace=False):
    from concourse.bass_utils import run_bass_kernel_spmd

    obs = np.asarray(obs, np.float32)
    action = np.asarray(action, np.float32)
    idx_wrap, dstc, chunks_wA, chunks_wB, dis = _prep_graph(np.asarray(edge_index))

    key = (tuple(chunks_wA.tolist()), tuple(chunks_wB.tolist()))
    if key not in _CACHE:
        _CACHE[key] = _build(chunks_wA, chunks_wB)
    nc = _CACHE[key]

    # x table in the remapped (core-interleaved) row order
    x = np.concatenate([obs, action], axis=1) * dis[:, None]
    perm = _remap(np.arange(N_NODES))
    x_dis = np.zeros((N_NODES, XCOLS), BF16)
    x_dis[perm, :OBS_DIM + ACT_DIM] = x.astype(BF16)

    w1p = np.zeros((XCOLS, HID), np.float32)
    w1p[:OBS_DIM + ACT_DIM, :] = np.asarray(w_g1, np.float32)
    bpk = np.zeros((P, 6 * P + 2), BF16)
    bpk[:, 0:P] = np.broadcast_to(np.arange(P, dtype=np.float32)[None, :], (P, P)).astype(BF16)
    bpk[:, P:2 * P] = np.eye(P, dtype=BF16)
    bpk[:, 2 * P:3 * P] = w1p.astype(BF16)
    bpk[:, 3 * P:4 * P] = np.asarray(w_g2, BF16)
    bpk[:, 4 * P:5 * P] = np.asarray(w_q1a, BF16)
    bpk[:, 5 * P:6 * P] = np.asarray(w_q2a, BF16)
    bpk[:, 6 * P:6 * P + 1] = np.asarray(w_q1b, np.float32).reshape(P, 1).astype(BF16)
    bpk[:, 6 * P + 1:6 * P + 2] = np.asarray(w_q2b, np.float32).reshape(P, 1).astype(BF16)
    fpk = np.zeros((P, P + 6), np.float32)
    fpk[:, 0:P] = np.eye(P, dtype=np.float32)
    fpk[:, P + 0] = np.asarray(b_g1, np.float32).reshape(P)
    fpk[:, P + 1] = np.asarray(b_g2, np.float32).reshape(P)
    fpk[:, P + 2] = np.asarray(b_q1a, np.float32).reshape(P)
    fpk[:, P + 3] = np.asarray(b_q2a, np.float32).reshape(P)
    fpk[:, P + 4] = float(np.asarray(b_q1b).reshape(-1)[0])
    fpk[:, P + 5] = float(np.asarray(b_q2b).reshape(-1)[0])

    in_maps = []
    for c in range(N_CORES):
        disp = np.zeros(NWIN * P, np.float32)
        disp[:BLK] = dis[c * BLK:(c + 1) * BLK]
        disb = np.broadcast_to(disp[None, :], (P, NWIN * P)).copy()
        in_maps.append(dict(
            x_dis=x_dis, idx=idx_wrap[c], dstc=dstc[c].astype(BF16),
            bpk=bpk, fpk=fpk, disb=disb,
        ))
    res = run_bass_kernel_spmd(nc, in_maps, core_ids=list(range(N_CORES)),
                               trace=_trace)
    q1 = np.concatenate([res.results[c]["q1"] for c in range(N_CORES)], axis=0)
    q2 = np.concatenate([res.results[c]["q2"] for c in range(N_CORES)], axis=0)
    kernel._last_exec_ns = res.exec_time_ns
    return (q1, q2)


# revision 34
# speedup vs baseline: 1.0856x; 1.0856x over previous
"""GCN critic (2x GCNConv + 2 MLP heads) on 8 trn2 NeuronCores.

Sharding: destination-node blocks of 1250 nodes per core. Edges bucketed by
dst window (128 dst nodes). conv1 gathers raw (dis-scaled) input features
(256B rows) from a replicated table; the w1 matmul is applied after the
segment-sum (linearity). conv2 gathers 256B rows of dis*relu(out1) from
AllGather'ed tables. Segment-sum is done per 128-edge chunk with a one-hot
matmul on the tensor engine (S[e,d] = (dst[e]==d)); msg.T @ S accumulates
feature-major segments in PSUM.

Key optimizations over the naive pipeline:
- one-hot S tiles are built once on DVE and shared by both convs;
- all dense matmuls (W1/W2/heads) run in bf16; heads use per-partition
  bias via the scalar engine plus a 1-column matmul for the final dot;
- windows are processed in pairs with one msg tile and one gather batch
  per pair, halving the number of partial (sub-1024-index) dma_gathers;
- the inter-conv AllGather is split in two; node ids are remapped
  host-side to the core-interleaved order the split AllGather produces;
  conv2's edges are further split per window into an A part (src row in
  the first AllGather half) and a B part, gathered from per-half tables
  so A gathers overlap the second collective;
- q outputs are transposed on the PE and written with 2 DMAs per head.
"""

import numpy as np
import ml_dtypes

BF16 = ml_dtypes.bfloat16
N_NODES = 10000
OBS_DIM = 30
ACT_DIM = 4
HID = 128
N_CORES = 8
BLK = N_NODES // N_CORES  # 1250 dst nodes per core
P = 128
NWIN = (BLK + P - 1) // P  # 10 windows per core (last is 98 wide)
GMAX = 2048  # max idx per dma_gather instruction
XCOLS = 128  # conv1 gather row (bf16): 34 used, pad to 256B
SPLIT = 5 * P  # rows 0..640 of each core strip go in AllGather #1
SPLIT2 = BLK - SPLIT  # 610


def _remap(node_ids):
    """Node id -> row in the split-AllGather table layout.

    AllGather #1 concatenates core strips' rows [0, SPLIT); #2 the rest.
    """
    c, r = node_ids // BLK, node_ids % BLK
    return np.where(r < SPLIT, c * SPLIT + r,
                    N_CORES * SPLIT + c * SPLIT2 + (r - SPLIT))


def _prep_graph(edge_index):
    """Host-side index preprocessing (the sharding step).

    Edges are bucketed by (core, dst window), then split into an A part
    (remapped src row < N_CORES*SPLIT, i.e. covered by AllGather #1) and a
    B part, each padded to 128-edge chunks. Chunk layout per window pair
    (w0, w1): [A(w0) | A(w1) | B(w0) | B(w1)] so that each pair needs one
    contiguous A gather batch and one contiguous B gather batch.
    """
    NA = N_CORES * SPLIT
    src = np.asarray(edge_index[0], dtype=np.int64)
    dst = np.asarray(edge_index[1], dtype=np.int64)
    loops = np.arange(N_NODES, dtype=np.int64)
    src = np.concatenate([src, loops])
    dst = np.concatenate([dst, loops])
    deg = np.bincount(dst, minlength=N_NODES).astype(np.float32)
    dis = (1.0 / np.sqrt(np.maximum(deg, 1.0))).astype(np.float32)

    src_m = _remap(src)
    isB = (src_m >= NA).astype(np.int64)
    win = (dst // BLK) * NWIN + (dst % BLK) // P  # (core, local window)
    nwin_g = N_CORES * NWIN
    order = np.lexsort((src_m, isB, win))
    src_s, dst_s, win_s, isB_s = src_m[order], dst[order], win[order], isB[order]
    grp = win_s * 2 + isB_s
    counts = np.bincount(grp, minlength=nwin_g * 2)
    starts = np.concatenate([[0], np.cumsum(counts)])[:-1]

    # common per-(window, part) chunk counts across cores (SPMD)
    chunks_wA = np.zeros(NWIN, dtype=np.int64)
    chunks_wB = np.zeros(NWIN, dtype=np.int64)
    for w in range(NWIN):
        chunks_wA[w] = (max(counts[(c * NWIN + w) * 2] for c in range(N_CORES)) + P - 1) // P
        chunks_wB[w] = (max(counts[(c * NWIN + w) * 2 + 1] for c in range(N_CORES)) + P - 1) // P

    tot_chunks = int(chunks_wA.sum() + chunks_wB.sum())
    tot_e = tot_chunks * P
    idx_all = np.zeros((N_CORES, tot_e), np.int16)
    dstc_all = np.full((N_CORES, tot_e), -1.0, np.float32)
    for c in range(N_CORES):
        off = 0
        for w0 in range(0, NWIN, 2):
            pair = [w for w in (w0, w0 + 1) if w < NWIN]
            for part, padidx, chunks_p in ((0, 0, chunks_wA), (1, NA, chunks_wB)):
                for w in pair:
                    g = (c * NWIN + w) * 2 + part
                    n = counts[g]
                    s0 = starts[g]
                    idx_all[c, off:off + chunks_p[w] * P] = padidx
                    idx_all[c, off:off + n] = src_s[s0:s0 + n].astype(np.int16)
                    dstc_all[c, off:off + n] = (dst_s[s0:s0 + n] - (c * BLK + w * P)).astype(np.float32)
                    off += chunks_p[w] * P
    # wrap idx: position i -> partition i%16, col i//16; replicate to 8 groups
    pos = np.arange(tot_e)
    idx_wrap = np.zeros((N_CORES, P, tot_e // 16), np.int16)
    for g in range(8):
        idx_wrap[:, g * 16 + pos % 16, pos // 16] = idx_all
    # dst cols: chunk k partition e
    dstc = dstc_all.reshape(N_CORES, tot_chunks, P).transpose(0, 2, 1).copy()
    return idx_wrap, dstc, chunks_wA, chunks_wB, dis


def _build(chunks_wA, chunks_wB):
    import concourse.bacc as bacc
    import concourse.mybir as mybir
    from concourse.tile import TileContext
    from concourse import library_config

    dt = mybir.dt
    act = mybir.ActivationFunctionType
    tot_chunks = int(chunks_wA.sum() + chunks_wB.sum())
    tot_e = tot_chunks * P

    nc = bacc.Bacc(None, target_bir_lowering=False, num_devices=N_CORES,
                   num_swdge_queues=4)
    # ---- inputs ----
    # bf16 pack cols: iota(P) | ident(P) | w1p(HID) | w2(HID) | wq1a(HID)
    #                | wq2a(HID) | w1bb(1) | w2bb(1)
    NB = 6 * P + 2
    # fp32 pack cols: identf(P) | b1(1) | b2(1) | a1c(1) | a2c(1) | bq(2)
    NF = P + 6
    x_dis = nc.dram_tensor("x_dis", [N_NODES, XCOLS], dt.bfloat16, kind="ExternalInput")
    idx_in = nc.dram_tensor("idx", [P, tot_e // 16], dt.int16, kind="ExternalInput")
    dstc_in = nc.dram_tensor("dstc", [P, tot_chunks], dt.bfloat16, kind="ExternalInput")
    bpk_in = nc.dram_tensor("bpk", [P, NB], dt.bfloat16, kind="ExternalInput")
    fpk_in = nc.dram_tensor("fpk", [P, NF], dt.float32, kind="ExternalInput")
    disb_in = nc.dram_tensor("disb", [P, NWIN * P], dt.float32, kind="ExternalInput")
    q1_out = nc.dram_tensor("q1", [BLK, 1], dt.float32, kind="ExternalOutput")
    q2_out = nc.dram_tensor("q2", [BLK, 1], dt.float32, kind="ExternalOutput")
    # per-AllGather-half gather tables (full-size so src ids stay absolute)
    x2d_fullA = nc.dram_tensor("x2d_fullA", [N_NODES, HID], dt.bfloat16,
                               kind="Internal", addr_space="Shared")
    x2d_fullB = nc.dram_tensor("x2d_fullB", [N_NODES, HID], dt.bfloat16,
                               kind="Internal", addr_space="Shared")

    with TileContext(nc) as tc:
        nc.gpsimd.load_library(library_config.mlp)
        with tc.tile_pool(name="const", bufs=1) as cp, \
             tc.tile_pool(name="msgp", bufs=3) as msgp, \
             tc.tile_pool(name="work", bufs=2) as wp, \
             tc.tile_pool(name="psum", bufs=4, space="PSUM") as pp, \
             tc.tile_pool(name="psum2", bufs=2, space="PSUM") as pp2, \
             tc.tile_pool(name="dram", bufs=1, space="DRAM") as dramp:

            # ---- load constants ----
            idx_t = cp.tile([P, tot_e // 16], dt.int16)
            nc.sync.dma_start(idx_t[:], idx_in[:])
            dstc_t = cp.tile([P, tot_chunks], dt.bfloat16)
            nc.scalar.dma_start(dstc_t[:], dstc_in[:])
            bpk_t = cp.tile([P, NB], dt.bfloat16)
            nc.scalar.dma_start(bpk_t[:], bpk_in[:])
            fpk_t = cp.tile([P, NF], dt.float32)
            nc.scalar.dma_start(fpk_t[:], fpk_in[:])
            disb_t = cp.tile([P, NWIN * P], dt.float32)
            nc.scalar.dma_start(disb_t[:], disb_in[:])

            iota_t = bpk_t[:, 0 * P:1 * P]
            ident_t = bpk_t[:, 1 * P:2 * P]
            w1_t = bpk_t[:, 2 * P:3 * P]
            w2_t = bpk_t[:, 3 * P:4 * P]
            wq1a_t = bpk_t[:, 4 * P:5 * P]
            wq2a_t = bpk_t[:, 5 * P:6 * P]
            w1bb_t = bpk_t[:, 6 * P:6 * P + 1]
            w2bb_t = bpk_t[:, 6 * P + 1:6 * P + 2]
            identf_t = fpk_t[:, 0:P]
            b1_t = fpk_t[:, P + 0:P + 1]
            b2_t = fpk_t[:, P + 1:P + 2]
            a1c_t = fpk_t[:, P + 2:P + 3]
            a2c_t = fpk_t[:, P + 3:P + 4]
            bq1_t = fpk_t[:, P + 4:P + 5]
            bq2_t = fpk_t[:, P + 5:P + 6]

            # split so AllGather #1's dependency covers only windows 0-4
            x2d_lo = dramp.tile([SPLIT, HID], dt.bfloat16)
            x2d_hi = dramp.tile([SPLIT2, HID], dt.bfloat16)
            q1_col = cp.tile([P, NWIN], dt.float32)
            q2_col = cp.tile([P, NWIN], dt.float32)

            qn = [0]

            # ---- one-hot S tiles: built once on DVE, shared by both convs ----
            S_all = cp.tile([P, tot_chunks, P], dt.bfloat16)
            SB = 34
            for c0 in range(0, tot_chunks, SB):
                nchunks = min(SB, tot_chunks - c0)
                dcol = dstc_t[:, c0:c0 + nchunks].rearrange(
                    "p (k o) -> p k o", o=1).broadcast_to([P, nchunks, P])
                irow = iota_t.rearrange(
                    "p (o d) -> p o d", o=1).broadcast_to([P, nchunks, P])
                nc.vector.tensor_tensor(out=S_all[:, c0:c0 + nchunks, :],
                                        in0=dcol, in1=irow,
                                        op=mybir.AluOpType.is_equal)

            def gather_span(table, msg, koff, c0, nchunks, ecols):
                """Issue dma_gathers for nchunks*P edges into msg at koff."""
                e0 = c0 * P
                n_left = nchunks * P
                off = 0
                while n_left > 0:
                    g = min(n_left, GMAX)
                    nc.gpsimd.dma_gather(
                        out_ap=msg[:, koff + off // P:koff + (off + g) // P, :],
                        in_ap=table[:],
                        idxs_ap=idx_t[:, (e0 + off) // 16:(e0 + off + g) // 16],
                        num_idxs=g, num_idxs_reg=g, elem_size=ecols,
                        queue_num=qn[0] % 4,
                    )
                    qn[0] += 1
                    off += g
                    n_left -= g

            def scatter_ranges(msg, ranges, ecols):
                """One-hot matmul segment sum over chunk ranges -> psum
                [ecols, P] feature-major. ranges: list of (koff, c0, n)."""
                seg = pp.tile([ecols, P], dt.float32, space="PSUM", tag="seg")
                total = sum(r[2] for r in ranges)
                i = 0
                for (koff, c0, n) in ranges:
                    for k in range(n):
                        nc.tensor.matmul(out=seg[:], lhsT=msg[:, koff + k, :],
                                         rhs=S_all[:, c0 + k, :],
                                         start=(i == 0), stop=(i == total - 1))
                        i += 1
                return seg

            def heads(x3, w):
                """q heads: h = relu(wqa.T @ x3 + a); q[d] = h.T @ wbb + bq."""
                for (wqa_t, ac_t, wbb_t, qcol, bqc) in (
                        (wq1a_t, a1c_t, w1bb_t, q1_col, bq1_t),
                        (wq2a_t, a2c_t, w2bb_t, q2_col, bq2_t)):
                    hp = pp2.tile([HID, P], dt.float32, space="PSUM", tag="mm")
                    nc.tensor.matmul(out=hp[:], lhsT=wqa_t, rhs=x3[:],
                                     start=True, stop=True)  # [f', d]
                    hr = wp.tile([HID, P], dt.bfloat16, tag="hr")
                    nc.scalar.activation(hr[:], hp[:], act.Relu,
                                         bias=ac_t, scale=1.0)
                    qp = pp2.tile([P, 1], dt.float32, space="PSUM", tag="qp")
                    nc.tensor.matmul(out=qp[:], lhsT=hr[:], rhs=wbb_t,
                                     start=True, stop=True)  # [d, 1]
                    nc.vector.tensor_scalar(out=qcol[:, w:w + 1], in0=qp[:],
                                            scalar1=bqc, scalar2=None,
                                            op0=mybir.AluOpType.add)

            # ================= conv1 =================
            # windows processed in pairs; chunk layout per pair is
            # [A(w0) | A(w1) | B(w0) | B(w1)] (A: src row < N_CORES*SPLIT)
            def pair_ranges(pair, c0):
                kA = [int(chunks_wA[w]) for w in pair]
                kB = [int(chunks_wB[w]) for w in pair]
                nA = sum(kA)
                r = {}
                offs = [0, kA[0], nA, nA + kB[0]]
                for j, w in enumerate(pair):
                    r[w] = [(offs[j], c0 + offs[j], kA[j]),
                            (offs[2 + j], c0 + offs[2 + j], kB[j])]
                return nA, sum(kB), r

            c0 = 0
            x2d_sb = cp.tile([P, NWIN, HID], dt.bfloat16)  # node-major x2d blocks
            for w0 in range(0, NWIN, 2):
              pair = [w for w in (w0, w0 + 1) if w < NWIN]
              nA, nB, pranges = pair_ranges(pair, c0)
              nch_pair = nA + nB
              msg = msgp.tile([P, nch_pair, XCOLS], dt.bfloat16, tag="msg")
              gather_span(x_dis, msg, 0, c0, nch_pair, XCOLS)
              for w in pair:
                wlen = min(P, BLK - w * P)
                segx = scatter_ranges(msg, pranges[w], XCOLS)  # psum fm
                segx_sb = wp.tile([XCOLS, P], dt.bfloat16, tag="segx")
                nc.scalar.copy(segx_sb[:], segx[:])
                o1 = pp2.tile([HID, P], dt.float32, space="PSUM", tag="mm")
                nc.tensor.matmul(out=o1[:], lhsT=w1_t, rhs=segx_sb[:],
                                 start=True, stop=True)  # [128f, 128d] fm
                t1 = wp.tile([HID, P], dt.float32, tag="t1")
                nc.vector.tensor_mul(t1[:], o1[:], disb_t[:, w * P:w * P + P])
                x2 = wp.tile([HID, P], dt.float32, tag="x2")
                nc.scalar.activation(x2[:], t1[:], act.Relu, bias=b1_t, scale=1.0)
                x2d = wp.tile([HID, P], dt.bfloat16, tag="x2d")
                nc.vector.tensor_mul(x2d[:], x2[:], disb_t[:, w * P:w * P + P])
                # transpose to node-major and stash
                x2d_tp = pp2.tile([P, HID], dt.bfloat16, space="PSUM", tag="mm")
                nc.tensor.transpose(out=x2d_tp[:], in_=x2d[:], identity=ident_t)
                nc.scalar.copy(x2d_sb[:, w, :], x2d_tp[:])
                if w < SPLIT // P:
                    nc.sync.dma_start(x2d_lo[w * P:w * P + wlen, :], x2d_sb[:wlen, w, :])
                else:
                    r0 = w * P - SPLIT
                    nc.sync.dma_start(x2d_hi[r0:r0 + wlen, :], x2d_sb[:wlen, w, :])
                if w == SPLIT // P - 1:  # rows [0, SPLIT) of the strip are done
                    nc.gpsimd.collective_compute(
                        "AllGather", mybir.AluOpType.bypass,
                        replica_groups=[list(range(N_CORES))],
                        ins=[x2d_lo[:].opt()],
                        outs=[x2d_fullA[0:N_CORES * SPLIT, :].opt()])
                    # scheduler fence: keep the collective's doorbell ahead
                    # of windows 5-9's gathers in the gpsimd stream so it
                    # fires mid-conv1 (ordering only, no semaphore waits)
                    tc.no_sync_barrier()
              c0 += nch_pair

            # ================= exchange (second half) =================
            nc.gpsimd.collective_compute(
                "AllGather", mybir.AluOpType.bypass,
                replica_groups=[list(range(N_CORES))],
                ins=[x2d_hi[:].opt()],
                outs=[x2d_fullB[N_CORES * SPLIT:N_NODES, :].opt()])

            # ================= conv2 + heads =================
            # A chunks gather from the first AllGather half's table (ready
            # before the second collective completes); B from the second
            c0 = 0
            for w0 in range(0, NWIN, 2):
              pair = [w for w in (w0, w0 + 1) if w < NWIN]
              nA, nB, pranges = pair_ranges(pair, c0)
              nch_pair = nA + nB
              msg = msgp.tile([P, nch_pair, HID], dt.bfloat16, tag="msg")
              gather_span(x2d_fullA, msg, 0, c0, nA, HID)
              gather_span(x2d_fullB, msg, nA, c0 + nA, nB, HID)
              for w in pair:
                seg2 = scatter_ranges(msg, pranges[w], HID)  # psum fm
                seg2_sb = wp.tile([HID, P], dt.bfloat16, tag="seg2")
                nc.scalar.copy(seg2_sb[:], seg2[:])
                o2 = pp2.tile([HID, P], dt.float32, space="PSUM", tag="mm")
                nc.tensor.matmul(out=o2[:], lhsT=w2_t, rhs=seg2_sb[:],
                                 start=True, stop=True)
                t2 = wp.tile([HID, P], dt.float32, tag="t2")
                nc.vector.tensor_mul(t2[:], o2[:], disb_t[:, w * P:w * P + P])
                x3 = wp.tile([HID, P], dt.bfloat16, tag="x3")
                nc.scalar.activation(x3[:], t2[:], act.Relu, bias=b2_t, scale=1.0)
                heads(x3, w)
              c0 += nch_pair

            # transpose q columns to window-major and write with 2 DMAs per head
            for (qcol, q_out) in ((q1_col, q1_out), (q2_col, q2_out)):
                qtp = pp2.tile([NWIN, P], dt.float32, space="PSUM", tag="mm")
                nc.tensor.transpose(out=qtp[:], in_=qcol[:], identity=identf_t)
                qsb = wp.tile([NWIN, P], dt.float32, tag="qsb")
                nc.scalar.copy(qsb[:], qtp[:])
                nfull = (BLK // P) * P  # 1152
                nc.sync.dma_start(
                    q_out[0:nfull, :].rearrange("(w p) o -> w (p o)", p=P),
                    qsb[0:BLK // P, :])
                nc.sync.dma_start(
                    q_out[nfull:BLK, :].rearrange("(w p) o -> w (p o)", w=1),
                    qsb[BLK // P:BLK // P + 1, 0:BLK - nfull])

    nc.compile()
    return nc


_CACHE = {}


def kernel(obs, action, edge_index,
           w_g1, b_g1, w_g2, b_g2,
           w_q1a, b_q1a, w_q1b, b_q1b,
           w_q2a, b_q2a, w_q2b, b_q2b, _trace=False):
    from concourse.bass_utils import run_bass_kernel_spmd

    obs = np.asarray(obs, np.float32)
    action = np.asarray(action, np.float32)
    idx_wrap, dstc, chunks_wA, chunks_wB, dis = _prep_graph(np.asarray(edge_index))

    key = (tuple(chunks_wA.tolist()), tuple(chunks_wB.tolist()))
    if key not in _CACHE:
        _CACHE[key] = _build(chunks_wA, chunks_wB)
    nc = _CACHE[key]

    # x table in the remapped (core-interleaved) row order
    x = np.concatenate([obs, action], axis=1) * dis[:, None]
    perm = _remap(np.arange(N_NODES))
    x_dis = np.zeros((N_NODES, XCOLS), BF16)
    x_dis[perm, :OBS_DIM + ACT_DIM] = x.astype(BF16)

    w1p = np.zeros((XCOLS, HID), np.float32)
    w1p[:OBS_DIM + ACT_DIM, :] = np.asarray(w_g1, np.float32)
    bpk = np.zeros((P, 6 * P + 2), BF16)
    bpk[:, 0:P] = np.broadcast_to(np.arange(P, dtype=np.float32)[None, :], (P, P)).astype(BF16)
    bpk[:, P:2 * P] = np.eye(P, dtype=BF16)
    bpk[:, 2 * P:3 * P] = w1p.astype(BF16)
    bpk[:, 3 * P:4 * P] = np.asarray(w_g2, BF16)
    bpk[:, 4 * P:5 * P] = np.asarray(w_q1a, BF16)
    bpk[:, 5 * P:6 * P] = np.asarray(w_q2a, BF16)
    bpk[:, 6 * P:6 * P + 1] = np.asarray(w_q1b, np.float32).reshape(P, 1).astype(BF16)
    bpk[:, 6 * P + 1:6 * P + 2] = np.asarray(w_q2b, np.float32).reshape(P, 1).astype(BF16)
    fpk = np.zeros((P, P + 6), np.float32)
    fpk[:, 0:P] = np.eye(P, dtype=np.float32)
    fpk[:, P + 0] = np.asarray(b_g1, np.float32).reshape(P)
    fpk[:, P + 1] = np.asarray(b_g2, np.float32).reshape(P)
    fpk[:, P + 2] = np.asarray(b_q1a, np.float32).reshape(P)
    fpk[:, P + 3] = np.asarray(b_q2a, np.float32).reshape(P)
    fpk[:, P + 4] = float(np.asarray(b_q1b).reshape(-1)[0])
    fpk[:, P + 5] = float(np.asarray(b_q2b).reshape(-1)[0])

    in_maps = []
    for c in range(N_CORES):
        disp = np.zeros(NWIN * P, np.float32)
        disp[:BLK] = dis[c * BLK:(c + 1) * BLK]
        disb = np.broadcast_to(disp[None, :], (P, NWIN * P)).copy()
        in_maps.append(dict(
            x_dis=x_dis, idx=idx_wrap[c], dstc=dstc[c].astype(BF16),
            bpk=bpk, fpk=fpk, disb=disb,
        ))
    res = run_bass_kernel_spmd(nc, in_maps, core_ids=list(range(N_CORES)),
                               trace=_trace)
    q1 = np.concatenate([res.results[c]["q1"] for c in range(N_CORES)], axis=0)
    q2 = np.concatenate([res.results[c]["q2"] for c in range(N_CORES)], axis=0)
    kernel._last_exec_ns = res.exec_time_ns
    return (q1, q2)
